# revision 30
# baseline (speedup 1.0000x reference)
"""Trainium2 Bass kernel for nn_CNN2D_simpleAttention (topk_masking).

Contract: kernel(**inputs) takes FULL inputs (features [32,512,56,56],
conv_w [64,512], conv_b [64]) and returns the full (x_out, sw, points)
tuple, computed on 8 NeuronCores with pure batch data-parallelism
(4 samples per core).

Per-core pipeline (4 samples):
  conv (fp32 PE, hw-col-tiled)  -> feats [64, 3136] (+bias on evac)
  Square (ScalarE) -> ones-matmul (PE) -> norm [1, 3136]
  DMA rearrange -> norm128 [128,25] (kth_largest) + norm_w [16,196]
  gpsimd.kth_largest -> T = 257th largest (exact data value)
  mask = norm > T  (exactly 256 ones)  -> sw output directly
  gpsimd.sparse_gather x2 -> compacted (value, hw) lists [16,16]
  rank via PE-broadcast + DVE compare-reduce (ties: lower hw first)
  indirect-DMA scatter (x, y, hw, 0) rows by rank -> stage DRAM
  gpsimd.ap_gather (2 samples/call) -> features in rank order
  PE transpose -> points rows; free-dim reduce -> x_out
"""

import os
import sys

for p in ("/opt/trn_rl_repo", "/root/.axon_site/_ro/trn_rl_repo"):
    if p not in sys.path:
        sys.path.append(p)

STAGE = int(os.environ.get("KSTAGE", "99"))
SUB = int(os.environ.get("KSUB", "9"))
KREP = int(os.environ.get("KREP", "1"))

import numpy as np

import concourse.bass as bass
import concourse.bacc as bacc
import concourse.mybir as mybir
import concourse.tile as tile
from concourse.bass_utils import run_bass_kernel_spmd

F32 = mybir.dt.float32
F32R = mybir.dt.float32r
CONV_R = os.environ.get("KCONVR", "0") == "1"
I16 = mybir.dt.int16
I32 = mybir.dt.int32
U32 = mybir.dt.uint32
AF = mybir.ActivationFunctionType
OP = mybir.AluOpType

B, C_IN, H, W = 32, 512, 56, 56
HW = H * W          # 3136
C = 64              # C_ENC
K = 256
NCORES = 8
SPC = B // NCORES   # samples per core = 4

BLK = 392           # hw block size; 3136 = 8 * 392, paired into 4 [128, 392] psums
NPAIR = 4
NCH = 7             # norm matmul chunks of 448
NCHW = 448
NEG = -1.0e30

# kth_largest quantile: need k_adj == K-1 == 255 with n_valid == 3136
_Q = 1.0 - (K - 0.5) / (HW - 1)
_omq = max(1, min(int(round((1.0 - _Q) * 4294967296)), 4294967295))
assert (_omq * (HW - 1)) >> 32 == K - 1, "quantile does not give k_adj=255"


def _build_core_module():
    nc = bacc.Bacc()

    fin = nc.dram_tensor("features4", [SPC * C_IN, HW], F32, kind="ExternalInput")
    w_in = nc.dram_tensor("conv_w", [C, C_IN], F32, kind="ExternalInput")
    b_in = nc.dram_tensor("conv_b", [C, 1], F32, kind="ExternalInput")

    x4 = nc.dram_tensor("x_out4", [SPC, C], F32, kind="ExternalOutput")
    sw4 = nc.dram_tensor("sw4", [SPC, 16, 196], F32, kind="ExternalOutput")
    pts4 = nc.dram_tensor("points4", [SPC, K, 3 + C], F32, kind="ExternalOutput")
    nf4 = nc.dram_tensor("nfound4", [SPC, 2, 1], U32, kind="ExternalOutput")
    dbg_norm = nc.dram_tensor("dbg_norm", [SPC, HW], F32, kind="ExternalOutput")
    dbg_feats = nc.dram_tensor("dbg_feats", [SPC, C, HW], F32, kind="ExternalOutput")
    dbg_sq = nc.dram_tensor("dbg_sq", [SPC, C, HW], F32, kind="ExternalOutput")

    # rank-sorted (x, y, hw, 0) rows, one staging table per sample
    stages = [
        nc.dram_tensor(f"stage{s}", [K, 4], F32, kind="Internal")
        for s in range(SPC)
    ]

    # constants baked into the NEFF
    ident = nc.inline_tensor(np.eye(C, dtype=np.float32), "ident64")
    ident2 = nc.inline_tensor(
        np.concatenate([np.eye(C, dtype=np.float32)] * 2, axis=0), "ident64x2"
    )
    ones_k = nc.inline_tensor(np.ones((C, 1), np.float32), "ones64")
    ident128 = nc.inline_tensor(np.eye(128, dtype=np.float32), "ident128")
    ones_b16 = nc.inline_tensor(np.ones((1, 16), np.float32), "ones1x16")
    ones_b128 = nc.inline_tensor(np.ones((1, 128), np.float32), "ones1x128")
    # norm_w[p, f] holds norm[196*p + f]; iota constant = hw + 1 in that layout
    iota_np = (196.0 * np.arange(16)[:, None] + np.arange(196)[None, :] + 1.0)
    iota1 = nc.inline_tensor(iota_np.astype(np.float32), "iota_w1")

    from contextlib import ExitStack

    with tile.TileContext(nc) as tc, ExitStack() as ctx:
        cst = ctx.enter_context(tc.tile_pool(name="cst", bufs=1))
        pf = ctx.enter_context(tc.tile_pool(name="pf", bufs=2))            # features in
        pfeat = ctx.enter_context(tc.tile_pool(name="pfeat", bufs=1))      # feats pair
        psq = ctx.enter_context(tc.tile_pool(name="psq", bufs=2))
        pnrm = ctx.enter_context(tc.tile_pool(name="pnrm", bufs=2))
        pnfl = ctx.enter_context(tc.tile_pool(name="pnfl", bufs=1))
        psmall = ctx.enter_context(tc.tile_pool(name="psmall", bufs=2))
        pvb = ctx.enter_context(tc.tile_pool(name="pvb", bufs=2))
        pout = ctx.enter_context(tc.tile_pool(name="pout", bufs=2))
        ps_conv = ctx.enter_context(tc.tile_pool(name="ps_conv", bufs=2, space="PSUM"))
        ps_trp = ctx.enter_context(tc.tile_pool(name="ps_trp", bufs=2, space="PSUM"))
        ps_misc = ctx.enter_context(tc.tile_pool(name="ps_misc", bufs=2, space="PSUM"))

        # ---- constants to SBUF ----
        ident_sb = cst.tile([C, C], F32, tag="ident")
        nc.sync.dma_start(ident_sb[:], ident[:])
        ident2_sb = cst.tile([2 * C, C], F32, tag="ident2")
        nc.sync.dma_start(ident2_sb[:], ident2[:])
        onesk_sb = cst.tile([C, 1], F32, tag="onesk")
        nc.sync.dma_start(onesk_sb[:], ones_k[:])
        ident128_sb = cst.tile([128, 128], F32, tag="ident128")
        nc.sync.dma_start(ident128_sb[:], ident128[:])
        ones16_sb = cst.tile([1, 16], F32, tag="ones16")
        nc.sync.dma_start(ones16_sb[:], ones_b16[:])
        ones128_sb = cst.tile([1, 128], F32, tag="ones128")
        nc.sync.dma_start(ones128_sb[:], ones_b128[:])
        iota_sb = cst.tile([16, 196], F32, tag="iota")
        nc.sync.dma_start(iota_sb[:], iota1[:])
        bias_sb = cst.tile([C, 1], F32, tag="bias")
        nc.sync.dma_start(bias_sb[:], b_in[:])

        # conv_w [64, 512] -> wT chunks [128, 64] via PE transpose
        w_sb = cst.tile([C, C_IN], F32, tag="w")
        nc.sync.dma_start(w_sb[:], w_in[:])
        wT = []
        for kk in range(4):
            ps_t = ps_misc.tile([128, C], F32, tag="misc")
            nc.tensor.transpose(
                out=ps_t[:], in_=w_sb[:, 128 * kk : 128 * (kk + 1)], identity=ident_sb[:]
            )
            wt_sb = cst.tile([128, C], F32, tag=f"wt{kk}")
            nc.vector.tensor_copy(wt_sb[:], ps_t[:])
            wT.append(wt_sb)

        xo_all = None
        if STAGE >= 8:
            xo_all = cst.tile([C, SPC], F32, tag="xoall")

        feat_pairs = {}

        for s_rep in range(KREP * SPC):
            s = s_rep % SPC
            # ---- load features for this sample ----
            ftiles = []
            for kk in range(4):
                ft = pf.tile([128, HW], F32, tag=f"fin{kk}")
                nc.sync.dma_start(ft[:], fin[s * C_IN + 128 * kk : s * C_IN + 128 * (kk + 1), :])
                ftiles.append(ft)

            # feats pair tile: samples (0,1) share one [128, HW] tile, (2,3) the next
            pair_id = s // 2
            half = s % 2
            if half == 0:
                fp_t = pfeat.tile([128, HW], F32, tag=f"fp{pair_id % 2}")
                feat_pairs[pair_id] = fp_t
            fp_t = feat_pairs[pair_id]
            feats = fp_t[64 * half : 64 * half + C, :]

            # ---- conv: out[c, hw] = sum_k w[c,k] f[k,hw], col-tiled over hw blocks ----
            for i in range(NPAIR):
                # two independent accumulation chains on different col groups;
                # separate PSUM banks (cols 0-391 vs 512-903) keep the
                # accumulation-group zero regions disjoint.
                ps_c = ps_conv.tile([128, 1024], F32, tag="conv")
                b0 = (2 * i) * BLK
                b1 = (2 * i + 1) * BLK
                for kk in range(4):
                    lhs_ap = wT[kk][:]
                    r0 = ftiles[kk][:, b0 : b0 + BLK]
                    r1 = ftiles[kk][:, b1 : b1 + BLK]
                    if CONV_R:
                        lhs_ap = lhs_ap.bitcast(F32R)
                        r0 = r0.bitcast(F32R)
                        r1 = r1.bitcast(F32R)
                    nc.tensor.matmul(
                        out=ps_c[0:64, 0:BLK], lhsT=lhs_ap, rhs=r0,
                        start=(kk == 0), stop=(kk == 3), tile_position=(0, 0),
                    )
                    nc.tensor.matmul(
                        out=ps_c[64:128, 512 : 512 + BLK], lhsT=lhs_ap, rhs=r1,
                        start=(kk == 0), stop=(kk == 3), tile_position=(0, 64),
                    )
                # evacuate + bias (per-channel scalar add)
                nc.vector.tensor_scalar(
                    out=feats[:, b0 : b0 + BLK], in0=ps_c[0:64, 0:BLK],
                    scalar1=bias_sb[:], scalar2=None, op0=OP.add,
                )
                nc.vector.tensor_scalar(
                    out=feats[:, b1 : b1 + BLK], in0=ps_c[64:128, 512 : 512 + BLK],
                    scalar1=bias_sb[:], scalar2=None, op0=OP.add,
                )

            # ---- squares + norm ----
            sq = psq.tile([C, HW], F32, tag="sq")
            nc.scalar.activation(out=sq[:], in_=feats[:, :], func=AF.Square)
            nc.sync.dma_start(dbg_feats[s], feats[:, :])
            nc.sync.dma_start(dbg_sq[s], sq[:])

            # transpose sq into [hw, ch] chunks so the channel sum is a true
            # sequential fp32 chain on the DVE (matches the reference's
            # reduce rounding, incl. tie collapses)
            sqT = psq.tile([128, 25 * C], F32, tag="sqT")
            nc.vector.memset(sqT[64:128, 24 * C : 25 * C], 0.0)
            nc.vector.memset(sqT[64:128, 24 * C : 24 * C + 1], NEG)
            for cchunk in range(25):
                lo = cchunk * 128
                wdt = min(128, HW - lo)
                ps_tq = ps_trp.tile([128, C], F32, tag="tr")
                nc.tensor.transpose(
                    out=ps_tq[0:wdt, :], in_=sq[:, lo : lo + wdt], identity=ident_sb[:]
                )
                nc.vector.tensor_copy(sqT[0:wdt, cchunk * C : cchunk * C + C], ps_tq[0:wdt, :])

            norm128 = pnrm.tile([128, 25], F32, tag="n128")
            nc.vector.tensor_reduce(
                out=norm128[:], in_=sqT[:].rearrange("p (c k) -> p c k", k=C),
                axis=mybir.AxisListType.X, op=OP.add,
            )

            # linearize: [128, 25] -> [25, 128] -> [1, 3136]
            ps_nt = ps_trp.tile([25, 128], F32, tag="tr")
            nc.tensor.transpose(out=ps_nt[:], in_=norm128[:], identity=ident128_sb[:])
            nrm_t = psmall.tile([25, 128], F32, tag="nrmt")
            nc.vector.tensor_copy(nrm_t[:], ps_nt[:])
            norm_flat = pnfl.tile([1, HW], F32, tag="nflat")
            nc.sync.dma_start(norm_flat[0:1, 0:3072], nrm_t[0:24, :])
            nc.sync.dma_start(norm_flat[0:1, 3072:3136], nrm_t[24:25, 0:64])
            norm_w = pnrm.tile([16, 196], F32, tag="nw")
            nc.sync.dma_start(norm_w[:], norm_flat[0:1, :])
            nc.sync.dma_start(dbg_norm[s : s + 1, :], norm_flat[0:1, :])

            if STAGE < 2:
                continue
            # ---- exact threshold: T = 257th largest ----
            kt = psmall.tile([1, 2], F32, tag="kt")
            nc.gpsimd.kth_largest(
                out_ap=kt[:], in_ap=norm128[:], n_per_lane=25, k=K, quantile=_Q
            )
            ps_t16 = ps_misc.tile([16, 1], F32, tag="misc")
            nc.tensor.matmul(
                out=ps_t16[:], lhsT=ones16_sb[:], rhs=kt[0:1, 1:2], start=True, stop=True
            )
            t16 = psmall.tile([16, 1], F32, tag="t16")
            nc.vector.tensor_copy(t16[:], ps_t16[:])

            if STAGE < 3:
                continue
            # ---- mask (this IS sw), masked values / indices ----
            mask = psmall.tile([16, 196], F32, tag="mask")
            nc.vector.tensor_scalar(
                out=mask[:], in0=norm_w[:], scalar1=t16[:], scalar2=None, op0=OP.is_gt
            )
            nc.sync.dma_start(sw4[s], mask[:])

            mnorm = psmall.tile([16, 196], F32, tag="mnorm")
            nc.vector.tensor_scalar(
                out=mnorm[:], in0=norm_w[:], scalar1=1.0, scalar2=None, op0=OP.add
            )
            nc.vector.tensor_tensor(out=mnorm[:], in0=mnorm[:], in1=mask[:], op=OP.mult)
            nc.vector.tensor_scalar(
                out=mnorm[:], in0=mnorm[:], scalar1=1.0, scalar2=None, op0=OP.subtract
            )
            mhw = psmall.tile([16, 196], F32, tag="mhw")
            nc.vector.tensor_tensor(out=mhw[:], in0=iota_sb[:], in1=mask[:], op=OP.mult)
            nc.vector.tensor_scalar(
                out=mhw[:], in0=mhw[:], scalar1=1.0, scalar2=None, op0=OP.subtract
            )

            if STAGE < 4:
                continue
            # ---- compact the 256 selected (value, hw) ----
            vals_c = psmall.tile([16, 16], F32, tag="valsc")
            hw_c = psmall.tile([16, 16], F32, tag="hwc")
            nfv = psmall.tile([1, 1], U32, tag="nfv")
            nfh = psmall.tile([1, 1], U32, tag="nfh")
            nc.gpsimd.sparse_gather(out=vals_c[:], in_=mnorm[:], num_found=nfv[:])
            nc.gpsimd.sparse_gather(out=hw_c[:], in_=mhw[:], num_found=nfh[:])
            nc.sync.dma_start(nf4[s, 0], nfv[:])
            nc.sync.dma_start(nf4[s, 1], nfh[:])

            if STAGE < 5:
                continue
            # ---- repack candidate lists ----
            v_row = psmall.tile([1, K], F32, tag="vrow")
            nc.sync.dma_start(v_row[:], vals_c[:])
            i_row = psmall.tile([1, K], F32, tag="irow")
            nc.sync.dma_start(i_row[:], hw_c[:])
            if SUB < 1:
                continue
            v_col = psmall.tile([128, 2], F32, tag="vcol")
            nc.sync.dma_start(v_col[:], vals_c[:])
            i_col = psmall.tile([128, 2], F32, tag="icol")
            nc.sync.dma_start(i_col[:], hw_c[:])

            if SUB < 2:
                continue
            ps_vb = ps_misc.tile([128, K], F32, tag="misc")
            nc.tensor.matmul(out=ps_vb[:], lhsT=ones128_sb[:], rhs=v_row[:], start=True, stop=True)
            vb = pvb.tile([128, K], F32, tag="vb")
            nc.vector.tensor_copy(vb[:], ps_vb[:])
            ps_ib = ps_misc.tile([128, K], F32, tag="misc")
            nc.tensor.matmul(out=ps_ib[:], lhsT=ones128_sb[:], rhs=i_row[:], start=True, stop=True)
            ib = pvb.tile([128, K], F32, tag="ib")
            nc.vector.tensor_copy(ib[:], ps_ib[:])

            if SUB < 3:
                continue
            # ---- rank = #greater + #(equal & lower hw)  (matches top_k order) ----
            rank_f = psmall.tile([128, 2], F32, tag="rankf")
            scr = pvb.tile([128, K], F32, tag="scr")
            scr2 = pvb.tile([128, K], F32, tag="scr2")
            for cc in range(2):
                rgt = psmall.tile([128, 1], F32, tag="rgt")
                nc.vector.tensor_scalar(
                    out=scr[:], in0=vb[:], scalar1=v_col[:, cc : cc + 1], scalar2=0.0,
                    op0=OP.is_gt, op1=OP.add, accum_out=rgt[:],
                )
                if SUB < 4:
                    nc.vector.tensor_copy(rank_f[:, cc : cc + 1], rgt[:])
                    continue
                nc.vector.tensor_scalar(
                    out=scr[:], in0=vb[:], scalar1=v_col[:, cc : cc + 1], scalar2=None,
                    op0=OP.is_equal,
                )
                nc.vector.tensor_scalar(
                    out=scr2[:], in0=ib[:], scalar1=i_col[:, cc : cc + 1], scalar2=None,
                    op0=OP.is_lt,
                )
                req = psmall.tile([128, 1], F32, tag="req")
                nc.vector.tensor_tensor(out=scr2[:], in0=scr[:], in1=scr2[:], op=OP.mult)
                nc.vector.tensor_reduce(
                    out=req[:], in_=scr2[:], axis=mybir.AxisListType.X, op=OP.add
                )
                nc.vector.tensor_tensor(
                    out=rank_f[:, cc : cc + 1], in0=rgt[:], in1=req[:], op=OP.add
                )

            rank_i = psmall.tile([128, 2], I32, tag="ranki")
            nc.vector.tensor_copy(rank_i[:], rank_f[:])

            if STAGE < 6:
                continue
            # ---- x, y from hw;  scatter (x, y, hw, 0) by rank ----
            stg = psmall.tile([128, 2, 4], F32, tag="stg")
            nc.vector.memset(stg[:, :, 3:4], 0.0)
            # y = hw // 56, robust to either trunc or round-to-nearest casts:
            # y0 = cast((hw+0.5)/56); x0 = hw - 56*y0; if x0 < 0: y -= 1, x += 56
            yf = psmall.tile([128, 2], F32, tag="yf")
            nc.vector.tensor_scalar(
                out=yf[:], in0=i_col[:], scalar1=0.5, scalar2=1.0 / 56.0,
                op0=OP.add, op1=OP.mult,
            )
            yi = psmall.tile([128, 2], I32, tag="yi")
            nc.vector.tensor_copy(yi[:], yf[:])
            y0 = psmall.tile([128, 2], F32, tag="y0")
            nc.vector.tensor_copy(y0[:], yi[:])
            x0 = psmall.tile([128, 2], F32, tag="x0")
            nc.vector.tensor_scalar(
                out=x0[:], in0=y0[:], scalar1=-56.0, scalar2=None, op0=OP.mult
            )
            nc.vector.tensor_tensor(out=x0[:], in0=i_col[:], in1=x0[:], op=OP.add)
            neg = psmall.tile([128, 2], F32, tag="neg")
            nc.vector.tensor_scalar(
                out=neg[:], in0=x0[:], scalar1=0.0, scalar2=None, op0=OP.is_lt
            )
            nc.vector.tensor_tensor(out=stg[:, :, 1], in0=y0[:], in1=neg[:], op=OP.subtract)
            nc.vector.tensor_scalar(
                out=neg[:], in0=neg[:], scalar1=56.0, scalar2=None, op0=OP.mult
            )
            nc.vector.tensor_tensor(out=stg[:, :, 0], in0=x0[:], in1=neg[:], op=OP.add)
            nc.vector.tensor_copy(stg[:, :, 2], i_col[:])

            for cc in range(2):
                nc.gpsimd.indirect_dma_start(
                    out=stages[s][:],
                    out_offset=bass.IndirectOffsetOnAxis(ap=rank_i[:, cc : cc + 1], axis=0),
                    in_=stg[:, cc, :],
                    in_offset=None,
                )

            # ---- after both samples of a pair are staged: gather features ----
            if STAGE >= 7 and half == 1:
                s0, s1 = 2 * pair_id, 2 * pair_id + 1
                idxt = pout.tile([128, 16], I16, tag="idxt")
                for blkc in range(8):
                    src = stages[s0 if blkc < 4 else s1]
                    src_r = src[:].rearrange("(f p) c -> p f c", p=16)
                    nc.gpsimd.dma_start(
                        idxt[16 * blkc : 16 * (blkc + 1), :], src_r[:, :, 2]
                    )
                if STAGE < 8:
                    continue
                fsel = pout.tile([128, K], F32, tag="fsel")
                nc.gpsimd.ap_gather(
                    out_ap=fsel[:], in_ap=fp_t[:], idxs_ap=idxt[:],
                    channels=128, num_elems=HW, d=1, num_idxs=K,
                )

                for ss, hh in ((s0, 0), (s1, 1)):
                    fs = fsel[64 * hh : 64 * hh + C, :]
                    # x_out: mean over the 256 selected pixels
                    xsum = psmall.tile([C, 1], F32, tag="xsum")
                    nc.vector.tensor_reduce(
                        out=xsum[:], in_=fs, axis=mybir.AxisListType.X, op=OP.add
                    )
                    nc.vector.tensor_scalar(
                        out=xo_all[:, ss : ss + 1], in0=xsum[:], scalar1=1.0 / K,
                        scalar2=None, op0=OP.mult,
                    )
                    # points: transpose features to [rank, C]
                    stp = pout.tile([128, 2, 3 + C], F32, tag="stp")
                    for blk in range(2):
                        ps_tr = ps_misc.tile([128, C], F32, tag="misc")
                        nc.tensor.transpose(
                            out=ps_tr[:], in_=fs[:, 128 * blk : 128 * (blk + 1)],
                            identity=ident2_sb[64 * hh : 64 * hh + C, :],
                        )
                        nc.vector.tensor_copy(stp[:, blk, 3:], ps_tr[:])
                    nc.vector.memset(stp[:, :, 2:3], 0.0)
                    st_r = stages[ss][:].rearrange("(blk p) c -> p blk c", p=128)
                    nc.sync.dma_start(stp[:, :, 0:2], st_r[:, :, 0:2])
                    pts_r = pts4[ss].rearrange("(blk p) c -> p blk c", p=128)
                    nc.sync.dma_start(pts_r[:], stp[:])

        # ---- x_out: [64, 4] -> [4, 64] ----
        if STAGE >= 8:
            ps_xo = ps_misc.tile([SPC, C], F32, tag="misc")
            nc.tensor.transpose(out=ps_xo[:], in_=xo_all[:], identity=ident_sb[:])
            xo_sb = psmall.tile([SPC, C], F32, tag="xosb")
            nc.vector.tensor_copy(xo_sb[:], ps_xo[:])
            nc.sync.dma_start(x4[:], xo_sb[:])

    if not nc.is_finalized():
        nc.finalize()
    return nc


_NC_CACHE = None


def kernel(features, conv_w, conv_b):
    global _NC_CACHE
    if _NC_CACHE is None:
        _NC_CACHE = _build_core_module()
    nc = _NC_CACHE

    features = np.ascontiguousarray(np.asarray(features, np.float32))
    conv_w = np.ascontiguousarray(np.asarray(conv_w, np.float32))
    conv_b = np.ascontiguousarray(np.asarray(conv_b, np.float32).reshape(C, 1))

    in_maps = []
    for core in range(NCORES):
        sl = features[core * SPC : (core + 1) * SPC].reshape(SPC * C_IN, HW)
        in_maps.append(
            {"features4": sl, "conv_w": conv_w, "conv_b": conv_b}
        )

    res = run_bass_kernel_spmd(nc, in_maps, core_ids=list(range(NCORES)))

    x_out = np.concatenate([r["x_out4"] for r in res.results], axis=0)
    sw = np.concatenate(
        [r["sw4"].reshape(SPC, HW) for r in res.results], axis=0
    ).reshape(B, 1, H, W)
    points = np.concatenate([r["points4"] for r in res.results], axis=0)
    nfound = np.concatenate([r["nfound4"] for r in res.results], axis=0)
    assert (nfound == K).all(), f"sparse_gather count mismatch: {nfound.reshape(-1)}"
    return (x_out, sw, points)
